# revision 1
# baseline (speedup 1.0000x reference)
"""Trainium2 Bass kernel for GPT-style multi-head causal self-attention.

Problem shapes (hardcoded): B=2, S=2048, D=1024, H=16, HD=64.
  qkv = x @ c_attn_w + c_attn_b ; split q,k,v ; per-head causal softmax(q k^T/8) v ;
  merge heads ; out = a @ c_proj_w + c_proj_b.

Sharding over 8 NeuronCores (tensor parallel over heads + sequence parallel
for the output projection):
  - Each core owns 2 heads: it gets the matching 384 columns of c_attn_w
    (q/k/v slices) and computes Q^T, K^T, V for its heads over ALL rows.
    hidden_states is fed pre-transposed (X^T, bf16) and replicated.
  - Attention runs in "transposed score" form: S^T[k,q] = K^T.T @ Q^T so the
    softmax denominator comes from a ones-augmented V matmul and no P
    transposes are needed. Causal masking skips strictly-upper blocks,
    trims fully-masked leading columns of diagonal blocks, and multiplies
    exp() by a 0/1 mask on the partial blocks.
  - An AllToAll converts head-sharding to row-sharding; each core computes
    its 512 rows of the output projection against the full c_proj_w.

Matmul dtypes: scores run in float32r (TF32-like; 1 cyc/row for moving
dim >= 256); QKV/PV/proj run in bf16. Overall rel err ~3e-3.
"""

import numpy as np

import concourse.bass as bass
import concourse.mybir as mybir
import concourse.tile as tile
from concourse import bacc
from concourse.bass_utils import run_bass_kernel_spmd
from concourse.masks import make_identity

NCORES = 8
B, S, D, H = 2, 2048, 1024, 16
HD = D // H            # 64
HPC = H // NCORES      # 2 heads per core
DH = HPC * HD          # 128 local head dims
R = B * S              # 4096 rows
RPC = R // NCORES      # 512 rows per core
QT_TILE = 512          # q tile (moving free dim)
KC = 128               # k chunk (psum partitions)
NQT = S // QT_TILE     # 4 q-tiles per batch
NKC = S // KC          # 16 k-chunks per batch
DCH = D // 128         # 8 contraction chunks over D
NRC = R // QT_TILE     # 8 row chunks
PAIRW = 2              # k-chunks per exp tile

F32 = mybir.dt.float32
BF16 = mybir.dt.bfloat16
MMDT = mybir.dt.float32r  # score matmul tile dtype
EVDT = BF16               # E/V tile dtype (post-softmax matmul)

_CACHED_NC = None


def _causal_masks() -> np.ndarray:
    """mask[d][kl, ql] = 1 if kl + 128*d <= ql else 0, for diagonal blocks."""
    import ml_dtypes
    m = np.zeros((4, KC, QT_TILE), dtype=np.float32)
    kl = np.arange(KC)[:, None]
    ql = np.arange(QT_TILE)[None, :]
    for d in range(4):
        m[d] = (kl + KC * d <= ql).astype(np.float32)
    return m.astype(ml_dtypes.bfloat16)


def _declare_io(nc):
    x_t = nc.dram_tensor("x_t", [D, R], BF16, kind="ExternalInput")
    w_qkv = nc.dram_tensor("w_qkv", [D, 3 * DH], BF16, kind="ExternalInput")
    b_qkv = nc.dram_tensor("b_qkv", [3 * DH], F32, kind="ExternalInput")
    w_p = nc.dram_tensor("w_p", [D, D], BF16, kind="ExternalInput")
    b_p = nc.dram_tensor("b_p", [D], F32, kind="ExternalInput")
    out = nc.dram_tensor("out", [RPC, D], F32, kind="ExternalOutput")
    masks_dram = nc.inline_tensor(_causal_masks(), name="causal_masks")
    return x_t, w_qkv, b_qkv, w_p, b_p, out, masks_dram


def build():
    nc = bacc.Bacc("TRN2", target_bir_lowering=False, debug=False,
                   num_devices=NCORES)
    io = _declare_io(nc)
    with tile.TileContext(nc) as tc:
        _build_body(nc, tc, *io)
    nc.compile()
    return nc


def _build_body(nc, tc, x_t, w_qkv, b_qkv, w_p, b_p, out, masks_dram,
                collectives=True):
    from contextlib import ExitStack
    ctx = ExitStack()
    with ctx:
        consts = ctx.enter_context(tc.tile_pool(name="consts", bufs=1))
        xpool = ctx.enter_context(tc.tile_pool(name="xpool", bufs=2))
        stage01 = ExitStack()
        tr_psum = stage01.enter_context(tc.tile_pool(name="tr_psum", bufs=2, space="PSUM"))
        dram = ctx.enter_context(tc.tile_pool(name="dram", bufs=1, space="DRAM"))

        ident_f32 = consts.tile([128, 128], F32)
        make_identity(nc, ident_f32[:])
        ident = consts.tile([128, 128], BF16)
        nc.vector.tensor_copy(ident[:], ident_f32[:])

        # ---- weights / biases to SBUF ----
        wqkv_sb = consts.tile([128, DCH, 3 * DH], BF16)
        nc.gpsimd.dma_start(wqkv_sb[:], w_qkv.ap().rearrange("(c p) n -> p c n", p=128))
        bq_sb = consts.tile([128, 1], F32)
        nc.sync.dma_start(bq_sb[:], b_qkv.ap()[0:DH][:, None])
        bk_sb = consts.tile([128, 1], F32)
        nc.sync.dma_start(bk_sb[:], b_qkv.ap()[DH:2 * DH][:, None])
        bv_sb = consts.tile([128, 1], F32)
        nc.sync.dma_start(bv_sb[:], b_qkv.ap()[2 * DH:3 * DH][:, None])

        # ---- stage 1: QKV projections, per 512-row chunk ----
        qkv_pool = ctx.enter_context(tc.tile_pool(name="qkv_pool", bufs=1))
        qkv_psum = stage01.enter_context(tc.tile_pool(name="qkv_psum", bufs=2, space="PSUM"))

        qt_c, kt_c, v_c = [], [], []
        for rc in range(NRC):
            qt_c.append(qkv_pool.tile([128, QT_TILE], MMDT, name=f"qt_{rc}"))
            kt_c.append(qkv_pool.tile([128, QT_TILE], MMDT, name=f"kt_{rc}"))
            # V rows-major per 128-row tile: [v0(64) ones v1(64) ones]
            v_c.append(qkv_pool.tile([128, 4, 130], EVDT, name=f"v_{rc}"))
        ones4_sb = consts.tile([128, 4], F32)
        nc.gpsimd.memset(ones4_sb[:], 1.0)

        for rc in range(NRC):
            xt_chunk = xpool.tile([128, DCH, QT_TILE], BF16, tag="xt_chunk", bufs=3)
            nc.sync.dma_start(
                xt_chunk[:],
                x_t.ap()[:, rc * QT_TILE:(rc + 1) * QT_TILE]
                .rearrange("(c p) n -> p c n", p=128))
            # Q^T and K^T: lhsT = w chunk [d,128], rhs = xt chunk [d,512]
            for which, bias_t, off in (("q", bq_sb, 0), ("k", bk_sb, DH)):
                ps = qkv_psum.tile([128, QT_TILE], F32, tag="qk_ps")
                for dc in range(DCH):
                    nc.tensor.matmul(
                        ps[:], wqkv_sb[:, dc, off:off + DH], xt_chunk[:, dc, :],
                        start=(dc == 0), stop=(dc == DCH - 1))
                dst = qt_c[rc] if which == "q" else kt_c[rc]
                nc.vector.tensor_scalar_add(dst[:], ps[:], bias_t[:])
            # V^T chunk then transpose to rows-major with bias bv
            vt_ps = qkv_psum.tile([128, QT_TILE], F32, tag="vt_ps")
            for dc in range(DCH):
                nc.tensor.matmul(
                    vt_ps[:], wqkv_sb[:, dc, 2 * DH:3 * DH], xt_chunk[:, dc, :],
                    start=(dc == 0), stop=(dc == DCH - 1))
            vt_sb = xpool.tile([128, QT_TILE], BF16, tag="vt_sb")
            nc.vector.tensor_scalar_add(vt_sb[:], vt_ps[:], bv_sb[:])
            nc.vector.tensor_copy(v_c[rc][:, :, 64], ones4_sb[:])
            nc.vector.tensor_copy(v_c[rc][:, :, 129], ones4_sb[:])
            for st in range(QT_TILE // 128):  # 4 sub-tiles of 128 rows
                tp = tr_psum.tile([128, 128], BF16, tag="tp")
                nc.tensor.transpose(tp[:], vt_sb[:, st * 128:(st + 1) * 128], ident[:])
                nc.vector.tensor_copy(v_c[rc][:, st, 0:64], tp[:, 0:64])
                nc.vector.tensor_copy(v_c[rc][:, st, 65:129], tp[:, 64:128])

        masks_sb = consts.tile([128, 4, QT_TILE], BF16)
        nc.sync.dma_start(masks_sb[:], masks_dram.ap().rearrange("d p n -> p d n"))
        wp_sb = consts.tile([128, DCH, D], BF16)
        nc.gpsimd.dma_start(wp_sb[:], w_p.ap().rearrange("(c p) n -> p c n", p=128))
        bp_bcast = consts.tile([128, D], F32)
        nc.sync.dma_start(bp_bcast[:], b_p.ap()[None, :].to_broadcast([128, D]))

        # ---- stage 2: attention per (batch, q-tile) ----
        stage01.close()
        stage2 = ExitStack()
        att_psum = stage2.enter_context(tc.tile_pool(name="att_psum", bufs=3, space="PSUM"))
        pv_psum = stage2.enter_context(tc.tile_pool(name="pv_psum", bufs=2, space="PSUM"))
        epool = ctx.enter_context(tc.tile_pool(name="epool", bufs=4))
        npool = ctx.enter_context(tc.tile_pool(name="npool", bufs=3))
        ndram = ctx.enter_context(tc.tile_pool(name="ndram", bufs=3, space="DRAM"))

        cc_in = dram.tile([NCORES, DH, QT_TILE], BF16)
        cc_out = dram.tile([NCORES, DH, QT_TILE], BF16)

        for b in range(B):
            for qq in range(NQT):
                dest = b * NQT + qq
                rcq = b * NQT + qq           # row chunk holding this q tile
                av_ps = [pv_psum.tile([128, QT_TILE], F32, tag="av_ps",
                                      name=f"av_ps_{b}_{qq}_{h}")
                         for h in range(HPC)]
                nkk = 4 * qq + 4
                npairs = nkk // PAIRW
                for j in range(npairs):
                    s2 = [att_psum.tile([128, PAIRW * QT_TILE], F32, tag="s2",
                                        name=f"s2_{b}_{qq}_{j}_{h}")
                          for h in range(HPC)]
                    # interleave heads so consecutive matmuls hit different
                    # PE row-groups (K=64 at partitions 0/64 run concurrently)
                    for ki in range(PAIRW):
                        kk = PAIRW * j + ki
                        rck = b * NQT + kk // 4
                        ko = (kk % 4) * KC
                        for h in range(HPC):
                            hsl = slice(h * HD, (h + 1) * HD)
                            nc.tensor.matmul(
                                s2[h][:, ki * QT_TILE:(ki + 1) * QT_TILE],
                                kt_c[rck][hsl, ko:ko + KC],
                                qt_c[rcq][hsl, :],
                                start=True, stop=True)
                    e_sb = [None, None]
                    for h in range(HPC):
                        et = epool.tile([128, PAIRW * QT_TILE], EVDT, tag="e_sb",
                                        name=f"e_{b}_{qq}_{j}_{h}")
                        nc.scalar.activation(
                            et[:], s2[h][:],
                            mybir.ActivationFunctionType.Exp, scale=0.125)
                        for ki in range(PAIRW):
                            d = PAIRW * j + ki - 4 * qq
                            if d >= 0:  # diagonal block: triangular mask
                                mo = KC * d
                                nc.vector.tensor_tensor(
                                    et[:, ki * QT_TILE + mo:(ki + 1) * QT_TILE],
                                    et[:, ki * QT_TILE + mo:(ki + 1) * QT_TILE],
                                    masks_sb[:, d, mo:], mybir.AluOpType.mult)
                        e_sb[h] = et
                    for h in range(HPC):
                        vcols = slice(0, 65) if h == 0 else slice(65, 130)
                        for ki in range(PAIRW):
                            kk = PAIRW * j + ki
                            rck = b * NQT + kk // 4
                            d = kk - 4 * qq
                            po = KC * d if d > 0 else 0
                            nc.tensor.matmul(
                                av_ps[h][0:65, po:QT_TILE],
                                v_c[rck][:, kk % 4, vcols],
                                e_sb[h][:, ki * QT_TILE + po:(ki + 1) * QT_TILE],
                                start=(kk == 0), stop=(kk == nkk - 1))
                # normalize: a = pv * (1/sumexp)  (v-bias already folded into V)
                for h in range(HPC):
                    # copy PSUM out promptly (two partition-0-based copies; a
                    # single [65,...] copy + recip at base partition 64
                    # miscomputes on HW) so av_ps frees for the next q-tile
                    se_sb = npool.tile([1, QT_TILE], F32, tag="se_sb")
                    nc.vector.tensor_copy(se_sb[:], av_ps[h][64:65, :])
                    au_sb = npool.tile([64, QT_TILE], F32, tag="au_sb",
                                       name=f"au_{b}_{qq}_{h}")
                    nc.vector.tensor_copy(au_sb[:], av_ps[h][0:64, :])
                    rec_sb = npool.tile([1, QT_TILE], F32, tag="rec_sb")
                    nc.vector.reciprocal_approx_fast(rec_sb[:], se_sb[:])
                    rec_dram = ndram.tile([1, QT_TILE], F32, tag="rec_dram")
                    nc.sync.dma_start(rec_dram[:], rec_sb[:])
                    rb_sb = npool.tile([64, QT_TILE], F32, tag="rb_sb")
                    nc.sync.dma_start(rb_sb[:], rec_dram[:].to_broadcast([64, QT_TILE]))
                    a_sb = npool.tile([64, QT_TILE], BF16, tag="a_sb")
                    nc.vector.tensor_tensor(
                        a_sb[:], au_sb[:], rb_sb[:], mybir.AluOpType.mult)
                    nc.sync.dma_start(cc_in[dest, h * HD:(h + 1) * HD, :], a_sb[:])

        if collectives:
            nc.gpsimd.collective_compute(
                "AllToAll", mybir.AluOpType.bypass,
                replica_groups=[list(range(NCORES))],
                ins=[cc_in.opt()], outs=[cc_out.opt()],
            )
        else:
            for r in range(NCORES):
                nc.sync.dma_start(cc_out[r], cc_in[r])

        # ---- stage 3: output projection on own 512 rows ----
        stage2.close()
        opool = ctx.enter_context(tc.tile_pool(name="opool", bufs=2))
        opsum = ctx.enter_context(tc.tile_pool(name="opsum", bufs=8, space="PSUM"))
        o_ps = [opsum.tile([128, QT_TILE], F32, tag="o_ps", name=f"o_ps_{i}")
                for i in range(8)]
        for kc in range(NCORES):              # contraction chunk = source core
            ach = opool.tile([128, QT_TILE], BF16, tag="ach", bufs=3,
                             name=f"ach_{kc}")
            nc.sync.dma_start(ach[:], cc_out[kc])
            for mt in range(RPC // 128):      # 4 row tiles
                for nt in range(D // QT_TILE):  # 2 dout tiles
                    nc.tensor.matmul(
                        o_ps[mt * 2 + nt][:], ach[:, mt * 128:(mt + 1) * 128],
                        wp_sb[:, kc, nt * QT_TILE:(nt + 1) * QT_TILE],
                        start=(kc == 0), stop=(kc == NCORES - 1))
        for mt in range(RPC // 128):
            for nt in range(D // QT_TILE):
                o_sb = opool.tile([128, QT_TILE], F32, tag="o_sb")
                nc.vector.tensor_tensor(
                    o_sb[:], o_ps[mt * 2 + nt][:],
                    bp_bcast[:, nt * QT_TILE:(nt + 1) * QT_TILE],
                    mybir.AluOpType.add)
                nc.sync.dma_start(
                    out.ap()[mt * 128:(mt + 1) * 128,
                             nt * QT_TILE:(nt + 1) * QT_TILE], o_sb[:])


def _shard_inputs(hidden_states, c_attn_w, c_attn_b, c_proj_w, c_proj_b):
    import ml_dtypes
    X = np.asarray(hidden_states, dtype=np.float32).reshape(R, D)
    x_t = np.ascontiguousarray(X.T).astype(ml_dtypes.bfloat16)
    w_p = np.ascontiguousarray(c_proj_w).astype(ml_dtypes.bfloat16)
    b_p = np.ascontiguousarray(c_proj_b).astype(np.float32)
    in_maps = []
    for c in range(NCORES):
        cols = []
        for part in range(3):
            lo = part * D + c * DH
            cols.append(np.arange(lo, lo + DH))
        cols = np.concatenate(cols)
        in_maps.append({
            "x_t": x_t,
            "w_qkv": np.ascontiguousarray(c_attn_w[:, cols]).astype(ml_dtypes.bfloat16),
            "b_qkv": np.ascontiguousarray(c_attn_b[cols]).astype(np.float32),
            "w_p": w_p,
            "b_p": b_p,
        })
    return in_maps


def _get_nc():
    global _CACHED_NC
    if _CACHED_NC is None:
        _CACHED_NC = build()
    return _CACHED_NC


def kernel(hidden_states, c_attn_w, c_attn_b, c_proj_w, c_proj_b):
    nc = _get_nc()
    in_maps = _shard_inputs(hidden_states, c_attn_w, c_attn_b,
                            c_proj_w, c_proj_b)
    res = run_bass_kernel_spmd(nc, in_maps, core_ids=list(range(NCORES)))
    full = np.concatenate([res.results[c]["out"] for c in range(NCORES)], axis=0)
    return full.reshape(B, S, D).astype(np.float32)

